# revision 77
# baseline (speedup 1.0000x reference)
"""Depthwise 8192-tap temporal conv (NoRollCaTentLayer) on 8 TRN2 cores.

v5: FFT-by-matmul (L=8192 = 128x64 Cooley-Tukey), all-bf16 data path,
Hermitian half-spectrum, 5-round software-pipelined with hand-interleaved
issue order and per-phase PSUM rings.

Structure per round (16 channels): per-channel PE-stationary F2 (makes the
forward transform transpose-free), DVE twiddle, F1; pointwise X*conj(W) on
the k2-half only -- the upper half of the pair-packed spectrum is
reconstructed via an anti-diagonal J-reversal matmul (+/-1-scaled Act
copies), with the packing combines folded into the I1/J matmul
accumulation planes. Inverse: I1, inverse twiddle, PE transposes (u<->k2),
I2. Weight norms computed inline (square on GPSIMD, reduce on DVE,
partition-sum via ones-matmul); bias enters as a DC-term injection;
relu*EI collapses into one DVE clamp with per-core bounds (EI is uniform
per core: channels 768+ = cores 6,7 exactly). x and w live resident in
SBUF in [b, c, a] bf16 layout (host pre-transposes/pre-shuffles); input
DMAs are chunked and spread across both HWDGE queues ordered by need-by
time. TimelineSim: ~140us vs 819us for the f32r/DRAM-transpose baseline.
"""

import os
import sys

sys.path.insert(0, "/opt/trn_rl_repo")

import numpy as np

import concourse.bacc as bacc
import concourse.mybir as mybir
import concourse.tile as tile
from concourse.bass_utils import run_bass_kernel_spmd

T, C, FW = 4096, 1024, 8192
L = 8192
NUM_INH = 256
EPS = 1e-8
NCORES = 8
CPC = C // NCORES          # 128 channels per core
CH = 16                    # channels per round
R = CPC // CH              # 8 rounds
PAIRS = CH // 2

F32 = mybir.dt.float32
BF16 = mybir.dt.bfloat16
NPBF16 = mybir.dt.np(BF16)
KH = 33                    # half spectrum: k2 in [0, 32]

AX = mybir.AxisListType
ALU = mybir.AluOpType
ACTF = mybir.ActivationFunctionType


def _consts():
    a_ = np.arange(128)
    b_ = np.arange(64)
    k1_ = np.arange(128)
    k2_ = np.arange(64)
    u_ = np.arange(128)
    v_ = np.arange(32)

    def bf(m):
        return m.astype(np.float32).astype(NPBF16)

    F2 = np.exp(-2j * np.pi * np.outer(b_, k2_) / 64)           # [64, 64]
    TW = np.exp(-2j * np.pi * np.outer(a_, k2_) / L)            # [128, 64]
    F1M = np.exp(-2j * np.pi * np.outer(a_, k1_) / 128)         # [128, 128]
    I1M = np.exp(+2j * np.pi * np.outer(k1_, u_) / 128)         # [128, 128]
    ITW = np.exp(+2j * np.pi * np.outer(u_, k2_) / L) / L       # [128, 64]
    I2M = np.exp(+2j * np.pi * np.outer(k2_, v_) / 64)          # [64, 32]

    # half spectrum: only k2 in [0, KH)
    f2xc = bf(np.stack([F2.real[:32, :KH], F2.imag[:32, :KH]], axis=1))
    f2wc = bf(np.stack([F2.real[:, :KH], F2.imag[:, :KH]], axis=1))
    twc16 = bf(np.repeat(
        np.stack([TW.real[:, :KH], TW.imag[:, :KH]], axis=1)[:, :, :, None],
        CH, axis=3))
    jrev0 = np.zeros((128, 128), np.float32)
    jrev0[np.arange(128), 127 - np.arange(128)] = 1.0
    jrev = bf(np.stack([jrev0, -jrev0], axis=1))                 # [128,2,128]
    f1c = bf(np.stack([F1M.real, F1M.imag, -F1M.imag], axis=1))  # [128,3,128]
    i1c = bf(np.stack([I1M.real, I1M.imag, -I1M.imag, -I1M.real],
                      axis=1))                                   # [128,4,128]
    itw8 = bf(np.repeat(
        np.stack([ITW.real, ITW.imag], axis=1)[:, :, None, :], PAIRS, axis=2))
    i2c = bf(np.stack([I2M.real, I2M.imag, -I2M.imag], axis=1))  # [64, 3, 32]
    idt = bf(np.eye(128))
    ones1 = bf(np.ones((1, 128)))
    ones64 = bf(np.ones((64, 1)))
    return {
        "f2xc": f2xc, "f2wc": f2wc, "twc16": twc16, "f1c": f1c,
        "i1c": i1c, "itw8": itw8, "i2c": i2c, "idt": idt, "ones1": ones1,
        "ones64": ones64, "jrev": jrev,
    }


def _build():
    nc = bacc.Bacc("TRN2", target_bir_lowering=False, debug=False,
                   num_devices=NCORES)
    xT_d = nc.dram_tensor("xT", [CPC, T], BF16, kind="ExternalInput")
    wsh_d = nc.dram_tensor("wsh", [CPC, FW], BF16, kind="ExternalInput")
    eis_d = nc.dram_tensor("eis", [1, 1], F32, kind="ExternalInput")
    bnds_d = nc.dram_tensor("bnds", [32, 2], F32, kind="ExternalInput")
    binj_d = nc.dram_tensor("binj", [1, 2, R, PAIRS], BF16,
                            kind="ExternalInput")
    f2x_d = nc.dram_tensor("f2xc", [32, 2, KH], BF16, kind="ExternalInput")
    f2w_d = nc.dram_tensor("f2wc", [64, 2, KH], BF16, kind="ExternalInput")
    tw_d = nc.dram_tensor("twc16", [128, 2, KH, CH], BF16,
                          kind="ExternalInput")
    jrev_d = nc.dram_tensor("jrev", [128, 2, 128], BF16,
                            kind="ExternalInput")
    f1_d = nc.dram_tensor("f1c", [128, 3, 128], BF16, kind="ExternalInput")
    i1_d = nc.dram_tensor("i1c", [128, 4, 128], BF16, kind="ExternalInput")
    itw_d = nc.dram_tensor("itw8", [128, 2, PAIRS, 64], BF16,
                           kind="ExternalInput")
    i2_d = nc.dram_tensor("i2c", [64, 3, 32], BF16, kind="ExternalInput")
    idt_d = nc.dram_tensor("idt", [128, 128], BF16, kind="ExternalInput")
    ones_d = nc.dram_tensor("ones1", [1, 128], BF16, kind="ExternalInput")
    ones64_d = nc.dram_tensor("ones64", [64, 1], BF16, kind="ExternalInput")
    out_d = nc.dram_tensor("out", [T, CPC], F32, kind="ExternalOutput")

    _rep = int(os.environ.get("BASS_CONV_REPEAT", "1"))
    NR = R * _rep

    with tile.TileContext(nc) as tc:
        with (
            tc.tile_pool(name="sb", bufs=1) as sb,
            tc.tile_pool(name="ps", bufs=8, space="PSUM") as pspool,
        ):
            # ---- constants to SBUF (small ones first, SP queue) ----
            f2xt = sb.tile([32, 2, KH], BF16, tag="c_f2x")
            nc.sync.dma_start(out=f2xt[:], in_=f2x_d.ap())
            f2wt = sb.tile([64, 2, KH], BF16, tag="c_f2w")
            nc.sync.dma_start(out=f2wt[:], in_=f2w_d.ap())
            jrevt = sb.tile([128, 128], BF16, tag="c_jrev")
            nc.sync.dma_start(out=jrevt[:], in_=jrev_d.ap())
            f1t = sb.tile([128, 3, 128], BF16, tag="c_f1")
            nc.sync.dma_start(out=f1t[:], in_=f1_d.ap())
            i1t = sb.tile([128, 3, 128], BF16, tag="c_i1")
            nc.sync.dma_start(out=i1t[:], in_=i1_d.ap())
            i2t = sb.tile([64, 3, 32], BF16, tag="c_i2")
            nc.sync.dma_start(out=i2t[:], in_=i2_d.ap())
            idt_t = sb.tile([128, 128], BF16, tag="c_idt")
            nc.sync.dma_start(out=idt_t[:], in_=idt_d.ap())
            onest = sb.tile([1, 128], BF16, tag="c_ones")
            nc.sync.dma_start(out=onest[:], in_=ones_d.ap())
            ones64t = sb.tile([64, 1], BF16, tag="c_ones64")
            nc.sync.dma_start(out=ones64t[:], in_=ones64_d.ap())
            eist = sb.tile([1, 1], F32, tag="c_eis")
            nc.sync.dma_start(out=eist[:], in_=eis_d.ap())
            bndst = sb.tile([32, 2], F32, tag="c_bnds")
            nc.sync.dma_start(out=bndst[:], in_=bnds_d.ap())
            binjt = sb.tile([1, 2, R, PAIRS], BF16, tag="c_binj")
            nc.sync.dma_start(out=binjt[:], in_=binj_d.ap())
            # ---- resident inputs: [b, c, a]; w on the Act queue (parallel
            # with x/consts), both chunked so round 0 starts early ----
            xT_bca = xT_d.ap().rearrange("c (b a) -> b c a", b=32)
            wsh_bca = wsh_d.ap().rearrange("c (b a) -> b c a", b=64)
            xv = sb.tile([32, CPC, 128], BF16, tag="xv")
            wv = sb.tile([64, CPC, 128], BF16, tag="wv")
            twt = sb.tile([128, 2, KH, CH], BF16, tag="c_tw")
            itwt = sb.tile([128, 2, PAIRS, 64], BF16, tag="c_itw")
            for h in range(4):
                hs = slice(32 * h, 32 * (h + 1))
                nc.scalar.dma_start(out=wv[:, hs], in_=wsh_bca[:, hs])
                nc.sync.dma_start(out=xv[:, hs], in_=xT_bca[:, hs])
            # big replicated twiddle consts after the inputs (Act queue)
            nc.scalar.dma_start(out=twt[:], in_=tw_d.ap())
            nc.scalar.dma_start(out=itwt[:], in_=itw_d.ap())

            # prefetch the Sqrt activation table during the input DMAs
            # (LoadActFuncSet is 1.28us and otherwise fires mid-fill)
            sqwarm = sb.tile([32, 1], F32, tag="sqwarm")
            nc.scalar.sqrt(sqwarm[:], bndst[:, 1:2])

            out_vu = out_d.ap().rearrange("(v u) c -> v u c", v=32)

            # per-round state carried between pipeline stages
            st = [dict() for _ in range(NR)]

            def s0_norm(r):
                """square + reduce -> rn, and relu'd w. Pool in steady
                state; DVE during pipeline ramp (Pool gates the ramp)."""
                c0 = (r % R) * CH
                csl = slice(c0, c0 + CH)
                d = st[r]
                eng = nc.vector
                sqv = sb.tile([64, CH, 128], BF16, tag="sqv", bufs=2)
                eng.tensor_mul(sqv[:], wv[:, csl], wv[:, csl])
                sp = sb.tile([64, CH, 64], BF16, tag="sqp", bufs=2)
                nc.vector.tensor_add(sp[:], sqv[:, :, 0:64], sqv[:, :, 64:128])
                nb3 = sb.tile([64, CH, 1], F32, tag="nb3", bufs=2)
                nc.vector.tensor_reduce(nb3[:], sp[:], AX.X, ALU.add)
                nbh = sb.tile([64, CH], BF16, tag="nbh", bufs=2)
                nc.scalar.copy(nbh[:], nb3[:, :, 0])
                nps = pspool.tile([1, CH], F32, tag="psf1", bufs=2)
                nc.tensor.matmul(nps[:], ones64t[:], nbh[:],
                                 start=True, stop=True)
                n1 = sb.tile([1, CH], F32, tag="n1", bufs=2)
                nc.scalar.sqrt(n1[:], nps[:])
                nc.vector.tensor_scalar_max(n1[:], n1[:], EPS)
                rn1 = sb.tile([1, CH], F32, tag="rn1", bufs=2)
                nc.vector.reciprocal(rn1[:], n1[:])
                nc.vector.tensor_scalar_mul(rn1[:], rn1[:], eist[0:1, 0:1])
                rn16 = sb.tile([1, CH], BF16, tag="rn16", bufs=2)
                nc.scalar.copy(rn16[:], rn1[:])
                rps = pspool.tile([128, CH], F32, tag="psf1", bufs=2)
                nc.tensor.matmul(rps[:], onest[:], rn16[:],
                                 start=True, stop=True)
                rnrep = sb.tile([128, CH], BF16, tag="rnrep", bufs=3)
                nc.scalar.copy(rnrep[:], rps[:])
                d["rnrep"] = rnrep
                wrl = sb.tile([64, CH, 128], BF16, tag="wrl", bufs=3)
                eng.tensor_scalar_max(wrl[:], wv[:, csl], 0.0)
                d["wrl"] = wrl

            def pw_pool(r):
                """two independent pointwise products on Pool."""
                d = st[r]
                X, WH = d["X"], d["WH"]
                hi = slice(PAIRS, CH)
                g6p = sb.tile([128, KH, PAIRS], BF16, tag="pw", bufs=18)
                g7p = sb.tile([128, KH, PAIRS], BF16, tag="pw", bufs=18)
                nc.gpsimd.tensor_mul(g6p[:], X[:, 0, :, hi], WH[:, 0, :, hi])
                nc.gpsimd.tensor_mul(g7p[:], X[:, 1, :, hi], WH[:, 1, :, hi])
                d["g6p"], d["g7p"] = g6p, g7p

            def pw_dve(r):
                """pointwise P = X*conj(WH) on the k2-half, pair-packed into
                PP (plus packing) and PM (minus packing, for the mirror);
                bias DC inject."""
                rr = r % R
                d = st[r]
                X, WH = d["X"], d["WH"]
                lo = slice(0, PAIRS)
                hi = slice(PAIRS, CH)
                ms = slice(1, 32)
                # lo-channel product: prl + i*pil
                p1 = sb.tile([128, KH, PAIRS], BF16, tag="pw", bufs=18)
                p2 = sb.tile([128, KH, PAIRS], BF16, tag="pw", bufs=18)
                prl = sb.tile([128, KH, PAIRS], BF16, tag="pw", bufs=18)
                nc.vector.tensor_mul(p1[:], X[:, 0, :, lo], WH[:, 0, :, lo])
                nc.vector.tensor_mul(p2[:], X[:, 1, :, lo], WH[:, 1, :, lo])
                nc.vector.tensor_add(prl[:], p1[:], p2[:])
                pil = sb.tile([128, KH, PAIRS], BF16, tag="pw", bufs=18)
                nc.vector.tensor_mul(p1[:], X[:, 1, :, lo], WH[:, 0, :, lo])
                nc.vector.tensor_mul(p2[:], X[:, 0, :, lo], WH[:, 1, :, lo])
                nc.vector.tensor_sub(pil[:], p1[:], p2[:])
                # hi-channel product: prh + i*pih (re via Pool tiles)
                prh = sb.tile([128, KH, PAIRS], BF16, tag="pw", bufs=18)
                nc.vector.tensor_add(prh[:], d["g6p"][:], d["g7p"][:])
                pih = sb.tile([128, KH, PAIRS], BF16, tag="pw", bufs=18)
                nc.vector.tensor_mul(p1[:], X[:, 1, :, hi], WH[:, 0, :, hi])
                nc.vector.tensor_mul(p2[:], X[:, 0, :, hi], WH[:, 1, :, hi])
                nc.vector.tensor_sub(pih[:], p1[:], p2[:])
                # bias via DC term on P_lo/P_hi directly (k2=0 column is
                # outside the PM mirror slice, so this stays exact)
                nc.vector.tensor_add(prl[0:1, 0:1, :], prl[0:1, 0:1, :],
                                     binjt[:, 0, rr:rr + 1, :])
                nc.vector.tensor_add(pil[0:1, 0:1, :], pil[0:1, 0:1, :],
                                     binjt[:, 1, rr:rr + 1, :])
                for t in ("prl", "pil", "prh", "pih"):
                    d[t] = locals()[t]
                # mirror: PP[k1, k2] = conj(PM[127-k1, 64-k2]) for k2 33..63,
                # PM = P_lo - i P_hi folded into the J-reversal matmuls
                pmr = pspool.tile([128, 2, 31, PAIRS], F32, tag="psinv", bufs=4)
                nc.tensor.matmul(pmr[:, 0], jrevt[:, 0], prl[:, ms],
                                 start=True, stop=False)
                nc.tensor.matmul(pmr[:, 0], jrevt[:, 0], pih[:, ms],
                                 start=False, stop=True)
                nc.tensor.matmul(pmr[:, 1], jrevt[:, 0], pil[:, ms],
                                 start=True, stop=False)
                nc.tensor.matmul(pmr[:, 1], jrevt[:, 1], prh[:, ms],
                                 start=False, stop=True)
                PPu = sb.tile([128, 2, 31, PAIRS], BF16, tag="PPu", bufs=2)
                nc.scalar.copy(PPu[:, 0], pmr[:, 0, ::-1, :])
                nc.scalar.activation(PPu[:, 1], pmr[:, 1, ::-1, :],
                                     ACTF.Copy, scale=-1.0)
                d["PPu"] = PPu

            def f2_mm(r, which):
                c0 = (r % R) * CH
                d = st[r]
                src = xv if which == "x" else d["wrl"]
                coff = c0 if which == "x" else 0
                fm = f2xt if which == "x" else f2wt
                pss = []
                for g in range(4):
                    ps4 = pspool.tile([128, 4, 2, KH], F32, tag="psf2", bufs=2)
                    for ci in range(4):
                        nc.tensor.matmul(ps4[:, ci], src[:, coff + 4 * g + ci],
                                         fm[:], start=True, stop=True)
                    pss.append(ps4)
                d["f2ps_" + which] = pss

            def f2_copy(r, which):
                d = st[r]
                dst = sb.tile([128, 2, KH, CH], BF16,
                              tag="yp" if which == "x" else "wp", bufs=2)
                for g, ps4 in enumerate(d["f2ps_" + which]):
                    nc.scalar.copy(dst[:, :, :, 4 * g:4 * g + 4],
                                   ps4[:].transpose((0, 2, 3, 1)))
                d["yp" if which == "x" else "wp"] = dst

            def tw_dve(r, which):
                """complex twiddle, in place."""
                d = st[r]
                yp = d["yp" if which == "x" else "wp"]
                t1 = sb.tile([128, KH, CH], BF16, tag="tw4", bufs=16)
                t2 = sb.tile([128, KH, CH], BF16, tag="tw4", bufs=16)
                t3 = sb.tile([128, KH, CH], BF16, tag="tw4", bufs=16)
                t4 = sb.tile([128, KH, CH], BF16, tag="tw4", bufs=16)
                nc.vector.tensor_mul(t1[:], yp[:, 0], twt[:, 0])
                nc.vector.tensor_mul(t2[:], yp[:, 1], twt[:, 1])
                nc.vector.tensor_mul(t3[:], yp[:, 0], twt[:, 1])
                nc.vector.tensor_mul(t4[:], yp[:, 1], twt[:, 0])
                nc.vector.tensor_sub(yp[:, 0], t1[:], t2[:])
                nc.vector.tensor_add(yp[:, 1], t3[:], t4[:])


            def f1_mm(r, which):
                d = st[r]
                yp = d["yp" if which == "x" else "wp"]
                pss = []
                for ksl in (slice(0, 17), slice(17, KH)):
                    kn = ksl.stop - ksl.start
                    prt = pspool.tile([128, 17, CH], F32, tag="psf1", bufs=2)
                    pit = pspool.tile([128, 17, CH], F32, tag="psf1", bufs=2)
                    pr_ = prt[:, :kn]
                    pi_ = pit[:, :kn]
                    nc.tensor.matmul(pr_, f1t[:, 0], yp[:, 0, ksl],
                                     start=True, stop=False)
                    nc.tensor.matmul(pr_, f1t[:, 2], yp[:, 1, ksl],
                                     start=False, stop=True)
                    nc.tensor.matmul(pi_, f1t[:, 1], yp[:, 0, ksl],
                                     start=True, stop=False)
                    nc.tensor.matmul(pi_, f1t[:, 0], yp[:, 1, ksl],
                                     start=False, stop=True)
                    pss.append((ksl, pr_, pi_))
                d["f1ps_" + which] = pss

            def f1_copy(r, which, dve=False):
                d = st[r]
                X = sb.tile([128, 2, KH, CH], BF16,
                            tag="X" if which == "x" else "WH",
                            bufs=2 if which == "x" else 3)
                for ksl, pr_, pi_ in d["f1ps_" + which]:
                    if dve:
                        nc.vector.tensor_copy(X[:, 0, ksl], pr_)
                        nc.vector.tensor_copy(X[:, 1, ksl], pi_)
                    else:
                        nc.scalar.copy(X[:, 0, ksl], pr_)
                        nc.scalar.copy(X[:, 1, ksl], pi_)
                d["X" if which == "x" else "WH"] = X

            def rn_apply(r):
                """WH *= rn*ei -- P*rn == X*conj(W*rn), applied in a slack
                slot so the norm chain is off the w-side critical path."""
                d = st[r]
                WH = d["WH"]
                rnb = d["rnrep"][:].unsqueeze(1).unsqueeze(1)\
                    .broadcast_to((128, 2, KH, CH))
                nc.vector.tensor_mul(WH[:], WH[:], rnb)

            def i1_mm(r):
                """I1 with the PP/PM packing combines folded into the
                accumulation: PP_re = prl - pih, PP_im = pil + prh (lower
                k2) plus the mirrored upper half PPu."""
                d = st[r]
                tt = {k: d[k].transpose((0, 2, 1))
                      for k in ("prl", "pil", "prh", "pih")}
                ppu_r = d["PPu"][:, 0].transpose((0, 2, 1))
                ppu_i = d["PPu"][:, 1].transpose((0, 2, 1))
                q_r = pspool.tile([128, PAIRS, 64], F32, tag="psinv", bufs=4)
                q_i = pspool.tile([128, PAIRS, 64], F32, tag="psinv", bufs=4)
                lo_r, lo_i = q_r[:, :, 0:KH], q_i[:, :, 0:KH]
                up_r, up_i = q_r[:, :, KH:64], q_i[:, :, KH:64]
                nc.tensor.matmul(lo_r, i1t[:, 0], tt["prl"],
                                 start=True, stop=False)
                nc.tensor.matmul(lo_r, i1t[:, 3], tt["pih"],
                                 start=False, stop=False)
                nc.tensor.matmul(lo_r, i1t[:, 2], tt["pil"],
                                 start=False, stop=False)
                nc.tensor.matmul(lo_r, i1t[:, 2], tt["prh"],
                                 start=False, stop=True)
                nc.tensor.matmul(up_r, i1t[:, 0], ppu_r,
                                 start=True, stop=False)
                nc.tensor.matmul(up_r, i1t[:, 2], ppu_i,
                                 start=False, stop=True)
                nc.tensor.matmul(lo_i, i1t[:, 1], tt["prl"],
                                 start=True, stop=False)
                nc.tensor.matmul(lo_i, i1t[:, 2], tt["pih"],
                                 start=False, stop=False)
                nc.tensor.matmul(lo_i, i1t[:, 0], tt["pil"],
                                 start=False, stop=False)
                nc.tensor.matmul(lo_i, i1t[:, 0], tt["prh"],
                                 start=False, stop=True)
                nc.tensor.matmul(up_i, i1t[:, 1], ppu_r,
                                 start=True, stop=False)
                nc.tensor.matmul(up_i, i1t[:, 0], ppu_i,
                                 start=False, stop=True)
                d["qps"] = (q_r, q_i)

            def i1_copy(r):
                d = st[r]
                q_r, q_i = d["qps"]
                Q = sb.tile([128, 2, PAIRS, 64], BF16, tag="Q", bufs=2)
                nc.scalar.copy(Q[:, 0], q_r[:])
                nc.scalar.copy(Q[:, 1], q_i[:])
                d["Q"] = Q

            def itw_dve(r):
                d = st[r]
                Q = d["Q"]
                q1 = sb.tile([128, PAIRS, 64], BF16, tag="qw", bufs=12)
                q2 = sb.tile([128, PAIRS, 64], BF16, tag="qw", bufs=12)
                q3 = sb.tile([128, PAIRS, 64], BF16, tag="qw", bufs=12)
                q4 = sb.tile([128, PAIRS, 64], BF16, tag="qw", bufs=12)
                nc.vector.tensor_mul(q1[:], Q[:, 0], itwt[:, 0])
                nc.vector.tensor_mul(q2[:], Q[:, 1], itwt[:, 1])
                nc.gpsimd.tensor_mul(q3[:], Q[:, 0], itwt[:, 1])
                nc.gpsimd.tensor_mul(q4[:], Q[:, 1], itwt[:, 0])
                nc.vector.tensor_sub(Q[:, 0], q1[:], q2[:])
                nc.vector.tensor_add(Q[:, 1], q3[:], q4[:])

            def tr_pe(r):
                d = st[r]
                Q = d["Q"]
                qt = sb.tile([64, 2, PAIRS, 128], BF16, tag="qt", bufs=2)
                for p in range(2):
                    trp = pspool.tile([64, PAIRS, 128], BF16, tag="psinv", bufs=4)
                    for pr in range(PAIRS):
                        nc.tensor.transpose(trp[:, pr], Q[:, p, pr], idt_t[:])
                    nc.scalar.copy(qt[:, p], trp[:])
                d["qt"] = qt

            def i2_pe(r):
                d = st[r]
                qt = d["qt"]
                zs = []
                for j, usl in ((0, slice(0, 64)), (1, slice(64, 128))):
                    z_r = pspool.tile([32, PAIRS, 64], F32, tag="psinv", bufs=4)
                    z_i = pspool.tile([32, PAIRS, 64], F32, tag="psinv", bufs=4)
                    nc.tensor.matmul(z_r[:], i2t[:, 0], qt[:, 0, :, usl],
                                     start=True, stop=False)
                    nc.tensor.matmul(z_r[:], i2t[:, 2], qt[:, 1, :, usl],
                                     start=False, stop=True)
                    nc.tensor.matmul(z_i[:], i2t[:, 1], qt[:, 0, :, usl],
                                     start=True, stop=False)
                    nc.tensor.matmul(z_i[:], i2t[:, 0], qt[:, 1, :, usl],
                                     start=False, stop=True)
                    zs.append((usl, z_r, z_i))
                d["zs"] = zs

            def clamp_dve(r):
                rr = r % R
                c0 = rr * CH
                csl = slice(c0, c0 + CH)
                d = st[r]
                outm = sb.tile([32, 128, CH], F32, tag="outm", bufs=2)
                for usl, z_r, z_i in d["zs"]:
                    for p, zz in ((0, z_r), (1, z_i)):
                        dst = outm[:, usl, PAIRS * p:PAIRS * (p + 1)]\
                            .transpose((0, 2, 1))
                        nc.vector.tensor_scalar(
                            dst, zz[:], bndst[:, 0:1], bndst[:, 1:2],
                            ALU.max, ALU.min)
                nc.sync.dma_start(out=out_vu[:, :, csl], in_=outm[:])

            # 5-round-deep pipeline, hand-interleaved issue order so each
            # engine's in-order queue matches data-readiness.
            for it in range(NR + 4):
                r3, r2, r1, r0, rb = it, it - 1, it - 2, it - 3, it - 4
                if 0 <= rb < NR:
                    i2_pe(rb)           # PE first: qt ready from last iter
                if it == 3 and 0 <= r0 < NR:
                    pw_pool(r0)     # pipeline-fill case
                if 0 <= r0 < NR:
                    pw_dve(r0)
                if 0 <= rb < NR:
                    clamp_dve(rb)
                if 0 <= r0 < NR:
                    i1_mm(r0)
                if 0 <= r2 < NR:
                    f2_mm(r2, "x")
                    f2_copy(r2, "x")
                if 0 <= r0 < NR:
                    i1_copy(r0)
                if 0 <= r2 < NR:
                    f2_mm(r2, "w")
                    f2_copy(r2, "w")
                if 0 <= r1 < NR:
                    tw_dve(r1, "x")
                    rn_apply(r1)
                    f1_mm(r1, "x")
                    f1_copy(r1, "x")
                    pw_pool(r1)
                if 0 <= r0 < NR:
                    itw_dve(r0)
                    tr_pe(r0)
                if 0 <= r2 < NR:
                    tw_dve(r2, "w")
                    f1_mm(r2, "w")
                    f1_copy(r2, "w")
                if 0 <= r3 < NR:
                    s0_norm(r3)

    nc.compile()
    if os.environ.get("BASS_CONV_SIMTIME", "") == "1":
        try:
            from concourse.timeline_sim import TimelineSim
            ts = TimelineSim(nc)
            ts.simulate()
            print(f"TimelineSim estimated exec: {ts.time:.0f} ns")
        except Exception as e:
            print(f"TimelineSim failed: {e}")
    return nc


_CACHE = {}


def kernel(x, w, b):
    if "nc" not in _CACHE:
        _CACHE["nc"] = _build()
        _CACHE["consts"] = _consts()
    nc = _CACHE["nc"]
    consts = _CACHE["consts"]

    x = np.asarray(x, dtype=np.float32)
    w = np.asarray(w, dtype=np.float32)
    b = np.asarray(b, dtype=np.float32)
    ei = np.concatenate([np.ones(C - NUM_INH, np.float32),
                         -np.ones(NUM_INH, np.float32)])

    # shuffled filter signal: s_w[l] = [w[4095:8191], 0, w[0:4095]]
    s_w = np.concatenate(
        [w[4095:8191], np.zeros((1, C), np.float32), w[0:4095]], axis=0)

    in_maps = []
    for i in range(NCORES):
        sl = slice(CPC * i, CPC * (i + 1))
        ei_i = float(ei[CPC * i])  # uniform per core
        big = 3.4e38
        bnds = np.empty((32, 2), np.float32)
        bnds[:, 0] = 0.0 if ei_i > 0 else -big
        bnds[:, 1] = big if ei_i > 0 else 0.0
        binj = np.zeros((1, 2, R, PAIRS), np.float32)
        for rr in range(R):
            for pl in range(2):
                for pr in range(PAIRS):
                    c = CPC * i + CH * rr + PAIRS * pl + pr
                    binj[0, pl, rr, pr] = float(L) * b[c] * ei[c]
        m = {
            "xT": np.ascontiguousarray(x[:, sl].T).astype(NPBF16),
            "wsh": np.ascontiguousarray(s_w[:, sl].T).astype(NPBF16),
            "eis": np.full((1, 1), ei_i, np.float32),
            "bnds": bnds,
            "binj": binj.astype(NPBF16),
        }
        m.update(consts)
        in_maps.append(m)

    trace = os.environ.get("BASS_CONV_TRACE", "") == "1"
    last_err = None
    for _attempt in range(3):
        try:
            res = run_bass_kernel_spmd(nc, in_maps, core_ids=list(range(NCORES)),
                                       trace=trace)
            break
        except Exception as e:  # transient NRT device errors happen under axon
            last_err = e
    else:
        raise last_err
    if trace and res.exec_time_ns is not None:
        print(f"HW exec time: {res.exec_time_ns} ns")
        kernel.last_exec_ns = res.exec_time_ns
    out = np.concatenate([res.results[i]["out"] for i in range(NCORES)], axis=1)
    return out
